# revision 7
# baseline (speedup 1.0000x reference)
"""Trainium2 Bass kernel for nn_Attention_Param_sharing_Kv_sharing.

Reference computation (per batch b, with x_b = x[b] viewed as [C=256, N=4096]):
    K   = w_qk' @ x_b + t_qk                  [16, N]    (BN folded into w', t)
    S   = K^T K                               [N, N]     (q == k shared -> symmetric)
    P   = exp(S)        (no max-subtraction; |S| < ~40 so fp32 exp is safe)
    r   = row sums of P = column sums of P    (symmetry)
    XXu^T[c,n] = sum_m V[c,m] P[m,n]          (= (attn @ V) * r, pre-normalized)
    out = (w_p' @ relu(XXu^T) + t_p (x) r) * (1/r)       [256, N]

Sharding: 8 cores = 4 batches x 2 column-halves of N.  The host permutes the
spatial axis per core so each core's own 2048 columns come first (attention
is permutation-equivariant over m when K and V are permuted together, and r
is permutation-invariant), which keeps the device program SPMD-uniform.

Symmetry of P means the P tiles computed in [m-partition, n-free] layout are
directly the P^T operand needed by the attn@V matmul -- no transposes.
r[n] (a partition-direction sum) comes from ones-vector matmuls on the PE.
The 1/r division is deferred past relu and the output projection (both
commute with the per-column scale), with t_p folded in as a rank-1 t_p (x) r
PSUM update.
"""

import numpy as np
import ml_dtypes

import concourse.bass as bass
import concourse.mybir as mybir
import concourse.tile as tile
from concourse import bacc
from concourse.bass import ts

F32 = mybir.dt.float32
F32R = mybir.dt.float32r
BF16 = mybir.dt.bfloat16

N_CORES = 8
B, C, H, W = 4, 256, 64, 64
N = H * W            # 4096
KD = 16              # qk dim
DH = 128             # value channels
EPS = 1e-5

NSH = N // 2         # 2048 n-columns per core
NBLK = 512           # n-block width
NBLOCKS = NSH // NBLK  # 4
MT = N // 128        # 32 m-tiles
ROUND = 2            # S m-tiles per exp round (2 psum banks, double-buffered)

_CACHE = {}


def _emit(nc, pools, dram):
    const, pbuf, work, outp, ps_s, ps_xx, ps_rrb, ps_pj = pools
    (xf_d, xb_d, wqkT_d, wvT_d, wpT_d, tqk_d, tv_d, tp_d, on1_d, out_d) = dram

    # ---- constants / weights ----
    xf = const.tile([128, 2, N], F32R, tag="xf")
    nc.sync.dma_start(out=xf, in_=xf_d.ap())
    xb = const.tile([128, 2, N], BF16, tag="xb")
    nc.sync.dma_start(out=xb, in_=xb_d.ap())
    wqkT = const.tile([128, 2, 128], F32R, tag="wqkT")
    nc.sync.dma_start(out=wqkT, in_=wqkT_d.ap())
    wvT = const.tile([128, 2, DH], BF16, tag="wvT")
    nc.sync.dma_start(out=wvT, in_=wvT_d.ap())
    wpT = const.tile([128, 2, 128], F32R, tag="wpT")
    nc.sync.dma_start(out=wpT, in_=wpT_d.ap())
    tqk = const.tile([128, 1], F32, tag="tqk")
    nc.sync.dma_start(out=tqk, in_=tqk_d.ap())
    tp = const.tile([1, 2, 128], F32R, tag="tp")
    nc.sync.dma_start(out=tp, in_=tp_d.ap())
    tvb = const.tile([128, DH], F32, tag="tvb")
    nc.sync.dma_start(
        out=tvb, in_=bass.AP(tensor=tv_d, offset=0, ap=[[0, 128], [1, DH]])
    )
    ones_bf = const.tile([128, 1], BF16, tag="ones_bf")
    nc.vector.memset(ones_bf, 1.0)
    ones1 = const.tile([1, 128], F32R, tag="ones1")
    nc.sync.dma_start(out=ones1, in_=on1_d.ap())

    # ---- K projection (replicated 4x across 32-row groups for S packing):
    # k_sb rows 32g+d (d<16) hold K[d, :]; rows 32g+16.. are zero.  ----
    k_sb = const.tile([128, N], F32R, tag="k_sb")
    for half in range(4):  # quarters of N, psum [128, 1024]
        kps = ps_s.tile([128, 2 * NBLK], F32, tag="s")
        for q in range(2):
            for cb in range(2):
                nc.tensor.matmul(
                    kps[:, ts(q, NBLK)],
                    wqkT[:, cb, :],
                    xf[:, cb, ts(half * 2 + q, NBLK)],
                    start=(cb == 0),
                    stop=(cb == 1),
                )
        nc.vector.tensor_scalar(
            out=k_sb[:, ts(half, 2 * NBLK)],
            in0=kps,
            scalar1=tqk,
            scalar2=None,
            op0=mybir.AluOpType.add,
        )

    # ---- V^T: VT[m, c] = sum_C x[C, m] wv'[c, C] + tv  -> bf16 ----
    vt_sb = const.tile([128, MT, DH], BF16, tag="vt_sb")
    for mi in range(MT):
        vps = ps_xx.tile([128, DH], F32, tag="xx")
        for cb in range(2):
            nc.tensor.matmul(
                vps,
                xb[:, cb, ts(mi, 128)],
                wvT[:, cb, :],
                start=(cb == 0),
                stop=(cb == 1),
            )
        nc.vector.tensor_add(vt_sb[:, mi, :], vps, tvb)

    # ---- main loop over this core's n-blocks (local = global) ----
    for j in range(NBLOCKS):
        p_sb = pbuf.tile([128, MT * NBLK], BF16, tag="p")
        xxps = ps_xx.tile([128, NBLK], F32, tag="xx")

        for k4 in range(MT // 4):
            sa = ps_s.tile([128, 2 * NBLK], F32, tag="s")
            sb = ps_s.tile([128, 2 * NBLK], F32, tag="s")
            # 4 concurrent S matmuls in distinct 32-row PE groups
            for q in range(4):
                mi = k4 * 4 + q
                dst = (sa if q < 2 else sb)[:, ts(q % 2, NBLK)]
                g = 32 * q
                nc.tensor.matmul(
                    dst,
                    k_sb[g:g + KD, ts(mi, 128)],
                    k_sb[g:g + KD, ts(j, NBLK)],
                    start=True,
                    stop=True,
                    tile_position=(g, 0),
                )
            # exp (psum -> sbuf bf16), 2 tiles per call
            nc.scalar.activation(
                out=p_sb[:, ts(2 * k4, 2 * NBLK)],
                in_=sa,
                func=mybir.ActivationFunctionType.Exp,
            )
            nc.scalar.activation(
                out=p_sb[:, ts(2 * k4 + 1, 2 * NBLK)],
                in_=sb,
                func=mybir.ActivationFunctionType.Exp,
            )
            # attn@V accumulation for this group's m-tiles
            for q in range(4):
                mi = k4 * 4 + q
                nc.tensor.matmul(
                    xxps,
                    vt_sb[:, mi, :],
                    p_sb[:, ts(mi, NBLK)],
                    start=(mi == 0),
                    stop=(mi == MT - 1),
                )

        # row sums r: ones-vector matmuls, stationary loaded once
        rps = ps_rrb.tile([1, NBLK], F32, tag="rrb")
        for mi in range(MT):
            nc.tensor.matmul(
                rps,
                ones_bf,
                p_sb[:, ts(mi, NBLK)],
                start=(mi == 0),
                stop=(mi == MT - 1),
            )

        # ---- epilogue for block j ----
        relu_sb = work.tile([128, NBLK], F32R, tag="relu")
        nc.vector.tensor_scalar(
            out=relu_sb,
            in0=xxps,
            scalar1=0.0,
            scalar2=None,
            op0=mybir.AluOpType.max,
        )
        r_sb = work.tile([1, NBLK], F32R, tag="r")
        nc.vector.tensor_copy(r_sb, rps)
        rinv_sb = work.tile([1, NBLK], F32R, tag="rinv")
        with nc.allow_low_precision(reason="fp32r matmul operand"):
            nc.vector.reciprocal(rinv_sb, r_sb)
        rbps = ps_rrb.tile([128, NBLK], F32, tag="rrb")
        nc.tensor.matmul(rbps, ones1, rinv_sb, start=True, stop=True)
        rb_sb = work.tile([128, NBLK], F32, tag="rb")
        nc.vector.tensor_copy(rb_sb, rbps)
        for h2 in range(2):
            pjps = ps_pj.tile([128, NBLK], F32, tag="pj")
            nc.tensor.matmul(
                pjps, wpT[:, h2, :], relu_sb, start=True, stop=False
            )
            nc.tensor.matmul(
                pjps, tp[:, h2, :], r_sb, start=False, stop=True
            )
            o_sb = outp.tile([128, NBLK], F32, tag="o")
            nc.vector.tensor_mul(o_sb, pjps, rb_sb)
            nc.sync.dma_start(out=out_d[h2, :, ts(j, NBLK)], in_=o_sb)


def build_nc(reps=1):
    key = ("nc", reps)
    if key in _CACHE:
        return _CACHE[key]

    nc = bacc.Bacc("TRN2", target_bir_lowering=False, debug=False)

    xf_d = nc.dram_tensor("xf", [128, 2, N], F32R, kind="ExternalInput")
    xb_d = nc.dram_tensor("xb", [128, 2, N], BF16, kind="ExternalInput")
    wqkT_d = nc.dram_tensor("wqkT", [128, 2, 128], F32R, kind="ExternalInput")
    wvT_d = nc.dram_tensor("wvT", [128, 2, DH], BF16, kind="ExternalInput")
    wpT_d = nc.dram_tensor("wpT", [128, 2, 128], F32R, kind="ExternalInput")
    tqk_d = nc.dram_tensor("tqk", [128, 1], F32, kind="ExternalInput")
    tv_d = nc.dram_tensor("tv", [1, DH], F32, kind="ExternalInput")
    tp_d = nc.dram_tensor("tp", [1, 2, 128], F32R, kind="ExternalInput")
    on1_d = nc.dram_tensor("on1", [1, 128], F32R, kind="ExternalInput")
    out_d = nc.dram_tensor("out", [2, 128, NSH], F32, kind="ExternalOutput")
    dram = (xf_d, xb_d, wqkT_d, wvT_d, wpT_d, tqk_d, tv_d, tp_d, on1_d, out_d)

    with tile.TileContext(nc) as tc:
        with (
            tc.tile_pool(name="const", bufs=1) as const,
            tc.tile_pool(name="pbuf", bufs=2) as pbuf,
            tc.tile_pool(name="work", bufs=2) as work,
            tc.tile_pool(name="outp", bufs=3) as outp,
            tc.tile_pool(name="ps_s", bufs=2, space="PSUM") as ps_s,
            tc.tile_pool(name="ps_xx", bufs=2, space="PSUM") as ps_xx,
            tc.tile_pool(name="ps_rrb", bufs=1, space="PSUM") as ps_rrb,
            tc.tile_pool(name="ps_pj", bufs=1, space="PSUM") as ps_pj,
        ):
            pools = (const, pbuf, work, outp, ps_s, ps_xx, ps_rrb, ps_pj)
            for _ in range(reps):
                _emit(nc, pools, dram)

    nc.compile()
    _CACHE[key] = nc
    return nc


def fold_bn(w, g, b, m, v):
    s = (g / np.sqrt(v + EPS)).astype(np.float32)
    return (w * s[:, None]).astype(np.float32), (b - m * s).astype(np.float32)


def make_in_maps(x, w_qk, g_qk, b_qk, m_qk, v_qk,
                 w_v, g_v, b_v, m_v, v_v, w_p, g_p, b_p, m_p, v_p):
    wqk_f, tqk_f = fold_bn(w_qk, g_qk, b_qk, m_qk, v_qk)   # [16,256], [16]
    wv_f, tv_f = fold_bn(w_v, g_v, b_v, m_v, v_v)          # [128,256], [128]
    wp_f, tp_f = fold_bn(w_p, g_p, b_p, m_p, v_p)          # [256,128], [256]

    # [128, 2, *]: partition dim first, C-half (or out-half) second.
    # wqkT replicated into 4 column groups of 32 (16 used + 16 zero) so the
    # S stage can row-pack 4 concurrent matmuls.
    wqkT_h = wqk_f.T.reshape(2, 128, KD).transpose(1, 0, 2)  # [128, 2, 16]
    wqkT = np.zeros((128, 2, 128), np.float32)
    for g in range(4):
        wqkT[:, :, 32 * g:32 * g + KD] = wqkT_h
    wqkT = np.ascontiguousarray(wqkT)
    wvT = np.ascontiguousarray(
        wv_f.T.reshape(2, 128, DH).transpose(1, 0, 2)).astype(ml_dtypes.bfloat16)
    wpT = np.ascontiguousarray(
        wp_f.T.reshape(128, 2, 128)).astype(np.float32)
    tqk = np.zeros((128, 1), np.float32)
    for g in range(4):
        tqk[32 * g:32 * g + KD, 0] = tqk_f
    tqk = np.ascontiguousarray(tqk)
    tv = tv_f.reshape(1, DH).astype(np.float32)
    tp = np.ascontiguousarray(tp_f.reshape(1, 2, 128)).astype(np.float32)

    xr = x.reshape(B, C, N).astype(np.float32)
    in_maps = []
    for c in range(N_CORES):
        b_, h_ = c // 2, c % 2
        # permute n so this core's half comes first
        if h_ == 0:
            xp = xr[b_]
        else:
            xp = np.concatenate([xr[b_][:, NSH:], xr[b_][:, :NSH]], axis=1)
        xp = np.ascontiguousarray(xp.reshape(2, 128, N).transpose(1, 0, 2))
        in_maps.append({
            "xf": xp.astype(np.float32),
            "xb": xp.astype(ml_dtypes.bfloat16),
            "wqkT": wqkT, "wvT": wvT, "wpT": wpT,
            "tqk": tqk, "tv": tv, "tp": tp,
            "on1": np.ones((1, 128), np.float32),
        })
    return in_maps


def assemble(results):
    """Per-core 'out' [2, 128, NSH] -> full [B, C, H, W]."""
    out = np.empty((B, C, N), np.float32)
    for c in range(N_CORES):
        b_, h_ = c // 2, c % 2
        o = results[c]["out"].reshape(C, NSH)
        out[b_][:, h_ * NSH:(h_ + 1) * NSH] = o
    return out.reshape(B, C, H, W)


def kernel(**inputs):
    from concourse.bass_utils import run_bass_kernel_spmd
    from concourse.bass_interp import get_hw_module

    inputs = dict(inputs)
    inputs.pop("key_v_input_reduction", None)  # unused by the reference
    nc = build_nc()
    in_maps = make_in_maps(**inputs)
    old_m = nc.m
    nc.m = get_hw_module(nc.m)
    try:
        res = run_bass_kernel_spmd(nc, in_maps, core_ids=list(range(N_CORES)))
    finally:
        nc.m = old_m
    return assemble(res.results)


# revision 9
# speedup vs baseline: 1.0953x; 1.0953x over previous
"""Trainium2 Bass kernel for nn_Attention_Param_sharing_Kv_sharing.

Reference computation (per batch b, with x_b = x[b] viewed as [C=256, N=4096]):
    K   = w_qk' @ x_b + t_qk                  [16, N]    (BN folded into w', t)
    S   = K^T K                               [N, N]     (q == k shared -> symmetric)
    P   = exp(S)        (no max-subtraction; |S| < ~40 so fp32 exp is safe)
    r   = row sums of P = column sums of P    (symmetry)
    XXu^T[c,n] = sum_m V[c,m] P[m,n]          (= (attn @ V) * r, pre-normalized)
    out = (w_p' @ relu(XXu^T) + t_p (x) r) * (1/r)       [256, N]

Sharding: 8 cores = 4 batches x 2 column-halves of N.  The host permutes the
spatial axis per core so each core's own 2048 columns come first (attention
is permutation-equivariant over m when K and V are permuted together, and r
is permutation-invariant), which keeps the device program SPMD-uniform.

Symmetry of P means the P tiles computed in [m-partition, n-free] layout are
directly the P^T operand needed by the attn@V matmul -- no transposes.
r[n] (a partition-direction sum) comes from ones-vector matmuls on the PE.
The 1/r division is deferred past relu and the output projection (both
commute with the per-column scale), with t_p folded in as a rank-1 t_p (x) r
PSUM update.
"""

import numpy as np
import ml_dtypes

import concourse.bass as bass
import concourse.mybir as mybir
import concourse.tile as tile
from concourse import bacc
from concourse.bass import ts

F32 = mybir.dt.float32
F32R = mybir.dt.float32r
BF16 = mybir.dt.bfloat16

N_CORES = 8
B, C, H, W = 4, 256, 64, 64
N = H * W            # 4096
KD = 16              # qk dim
DH = 128             # value channels
EPS = 1e-5

NSH = N // 2         # 2048 n-columns per core
NBLK = 512           # n-block width
NBLOCKS = NSH // NBLK  # 4
MT = N // 128        # 32 m-tiles
ROUND = 2            # S m-tiles per exp round (2 psum banks, double-buffered)

_CACHE = {}


def _emit(nc, pools, dram, pack_s=True):
    const, pbuf, work, outp, ps_s, ps_xx, ps_rrb, ps_pj = pools
    (xf_d, xb_d, wqkT_d, wvT_d, wpT_d, tqk_d, tv_d, tp_d, on1_d, out_d) = dram

    # ---- constants / weights ----
    xf = const.tile([128, 2, N], F32R, tag="xf")
    nc.sync.dma_start(out=xf, in_=xf_d.ap())
    xb = const.tile([128, 2, N], BF16, tag="xb")
    nc.sync.dma_start(out=xb, in_=xb_d.ap())
    wqkT = const.tile([128, 2, 128], F32R, tag="wqkT")
    nc.sync.dma_start(out=wqkT, in_=wqkT_d.ap())
    wvT = const.tile([128, 2, DH], BF16, tag="wvT")
    nc.sync.dma_start(out=wvT, in_=wvT_d.ap())
    wpT = const.tile([128, 2, 128], F32R, tag="wpT")
    nc.sync.dma_start(out=wpT, in_=wpT_d.ap())
    tqk = const.tile([128, 1], F32, tag="tqk")
    nc.sync.dma_start(out=tqk, in_=tqk_d.ap())
    tp = const.tile([1, 2, 128], F32R, tag="tp")
    nc.sync.dma_start(out=tp, in_=tp_d.ap())
    tvb = const.tile([128, DH], F32, tag="tvb")
    nc.sync.dma_start(
        out=tvb, in_=bass.AP(tensor=tv_d, offset=0, ap=[[0, 128], [1, DH]])
    )
    ones_bf = const.tile([128, 1], BF16, tag="ones_bf")
    nc.vector.memset(ones_bf, 1.0)
    ones1 = const.tile([1, 128], F32R, tag="ones1")
    nc.sync.dma_start(out=ones1, in_=on1_d.ap())

    # ---- K projection (replicated 4x across 32-row groups for S packing):
    # k_sb rows 32g+d (d<16) hold K[d, :]; rows 32g+16.. are zero.  ----
    k_sb = const.tile([128, N], F32R, tag="k_sb")
    for half in range(4):  # quarters of N, psum [128, 1024]
        kps = ps_s.tile([128, 2 * NBLK], F32, tag="s")
        for q in range(2):
            for cb in range(2):
                nc.tensor.matmul(
                    kps[:, ts(q, NBLK)],
                    wqkT[:, cb, :],
                    xf[:, cb, ts(half * 2 + q, NBLK)],
                    start=(cb == 0),
                    stop=(cb == 1),
                )
        nc.vector.tensor_scalar(
            out=k_sb[:, ts(half, 2 * NBLK)],
            in0=kps,
            scalar1=tqk,
            scalar2=None,
            op0=mybir.AluOpType.add,
        )

    # ---- V^T: VT[m, c] = sum_C x[C, m] wv'[c, C] + tv  -> bf16 ----
    vt_sb = const.tile([128, MT, DH], BF16, tag="vt_sb")
    for mi in range(MT):
        vps = ps_xx.tile([128, DH], F32, tag="xx")
        for cb in range(2):
            nc.tensor.matmul(
                vps,
                xb[:, cb, ts(mi, 128)],
                wvT[:, cb, :],
                start=(cb == 0),
                stop=(cb == 1),
            )
        nc.vector.tensor_add(vt_sb[:, mi, :], vps, tvb)

    # ---- main loop over this core's n-blocks (local = global) ----
    for j in range(NBLOCKS):
        p_sb = pbuf.tile([128, MT * NBLK], BF16, tag="p")
        xxps = ps_xx.tile([128, NBLK], F32, tag="xx")

        for k4 in range(MT // 4):
            sa = ps_s.tile([128, 2 * NBLK], F32, tag="s")
            sb = ps_s.tile([128, 2 * NBLK], F32, tag="s")
            # 4 concurrent S matmuls in distinct 32-row PE groups
            for q in range(4):
                mi = k4 * 4 + q
                dst = (sa if q < 2 else sb)[:, ts(q % 2, NBLK)]
                g = 32 * q if pack_s else 0
                nc.tensor.matmul(
                    dst,
                    k_sb[g:g + KD, ts(mi, 128)],
                    k_sb[g:g + KD, ts(j, NBLK)],
                    start=True,
                    stop=True,
                    tile_position=(g, 0),
                )
            # exp (psum -> sbuf bf16), 2 tiles per call
            nc.scalar.activation(
                out=p_sb[:, ts(2 * k4, 2 * NBLK)],
                in_=sa,
                func=mybir.ActivationFunctionType.Exp,
            )
            nc.scalar.activation(
                out=p_sb[:, ts(2 * k4 + 1, 2 * NBLK)],
                in_=sb,
                func=mybir.ActivationFunctionType.Exp,
            )
            # attn@V accumulation for this group's m-tiles
            for q in range(4):
                mi = k4 * 4 + q
                nc.tensor.matmul(
                    xxps,
                    vt_sb[:, mi, :],
                    p_sb[:, ts(mi, NBLK)],
                    start=(mi == 0),
                    stop=(mi == MT - 1),
                )

        # row sums r: ones-vector matmuls, stationary loaded once
        rps = ps_rrb.tile([1, NBLK], F32, tag="rrb")
        for mi in range(MT):
            nc.tensor.matmul(
                rps,
                ones_bf,
                p_sb[:, ts(mi, NBLK)],
                start=(mi == 0),
                stop=(mi == MT - 1),
            )

        # ---- epilogue for block j ----
        relu_sb = work.tile([128, NBLK], F32R, tag="relu")
        nc.vector.tensor_scalar(
            out=relu_sb,
            in0=xxps,
            scalar1=0.0,
            scalar2=None,
            op0=mybir.AluOpType.max,
        )
        r_sb = work.tile([1, NBLK], F32R, tag="r")
        nc.vector.tensor_copy(r_sb, rps)
        rinv_sb = work.tile([1, NBLK], F32R, tag="rinv")
        with nc.allow_low_precision(reason="fp32r matmul operand"):
            nc.vector.reciprocal(rinv_sb, r_sb)
        rbps = ps_rrb.tile([128, NBLK], F32, tag="rrb")
        nc.tensor.matmul(rbps, ones1, rinv_sb, start=True, stop=True)
        rb_sb = work.tile([128, NBLK], F32, tag="rb")
        nc.vector.tensor_copy(rb_sb, rbps)
        for h2 in range(2):
            pjps = ps_pj.tile([128, NBLK], F32, tag="pj")
            nc.tensor.matmul(
                pjps, wpT[:, h2, :], relu_sb, start=True, stop=False
            )
            nc.tensor.matmul(
                pjps, tp[:, h2, :], r_sb, start=False, stop=True
            )
            o_sb = outp.tile([128, NBLK], F32, tag="o")
            nc.vector.tensor_mul(o_sb, pjps, rb_sb)
            nc.sync.dma_start(out=out_d[h2, :, ts(j, NBLK)], in_=o_sb)


def build_nc(reps=1, pack_s=True):
    key = ("nc", reps, pack_s)
    if key in _CACHE:
        return _CACHE[key]

    nc = bacc.Bacc("TRN2", target_bir_lowering=False, debug=False)

    xf_d = nc.dram_tensor("xf", [128, 2, N], F32R, kind="ExternalInput")
    xb_d = nc.dram_tensor("xb", [128, 2, N], BF16, kind="ExternalInput")
    wqkT_d = nc.dram_tensor("wqkT", [128, 2, 128], F32R, kind="ExternalInput")
    wvT_d = nc.dram_tensor("wvT", [128, 2, DH], BF16, kind="ExternalInput")
    wpT_d = nc.dram_tensor("wpT", [128, 2, 128], F32R, kind="ExternalInput")
    tqk_d = nc.dram_tensor("tqk", [128, 1], F32, kind="ExternalInput")
    tv_d = nc.dram_tensor("tv", [1, DH], F32, kind="ExternalInput")
    tp_d = nc.dram_tensor("tp", [1, 2, 128], F32R, kind="ExternalInput")
    on1_d = nc.dram_tensor("on1", [1, 128], F32R, kind="ExternalInput")
    out_d = nc.dram_tensor("out", [2, 128, NSH], F32, kind="ExternalOutput")
    dram = (xf_d, xb_d, wqkT_d, wvT_d, wpT_d, tqk_d, tv_d, tp_d, on1_d, out_d)

    with tile.TileContext(nc) as tc:
        with (
            tc.tile_pool(name="const", bufs=1) as const,
            tc.tile_pool(name="pbuf", bufs=2) as pbuf,
            tc.tile_pool(name="work", bufs=2) as work,
            tc.tile_pool(name="outp", bufs=3) as outp,
            tc.tile_pool(name="ps_s", bufs=2, space="PSUM") as ps_s,
            tc.tile_pool(name="ps_xx", bufs=2, space="PSUM") as ps_xx,
            tc.tile_pool(name="ps_rrb", bufs=1, space="PSUM") as ps_rrb,
            tc.tile_pool(name="ps_pj", bufs=1, space="PSUM") as ps_pj,
        ):
            pools = (const, pbuf, work, outp, ps_s, ps_xx, ps_rrb, ps_pj)
            for _ in range(reps):
                _emit(nc, pools, dram, pack_s=pack_s)

    nc.compile()
    _CACHE[key] = nc
    return nc


def fold_bn(w, g, b, m, v):
    s = (g / np.sqrt(v + EPS)).astype(np.float32)
    return (w * s[:, None]).astype(np.float32), (b - m * s).astype(np.float32)


def make_in_maps(x, w_qk, g_qk, b_qk, m_qk, v_qk,
                 w_v, g_v, b_v, m_v, v_v, w_p, g_p, b_p, m_p, v_p):
    wqk_f, tqk_f = fold_bn(w_qk, g_qk, b_qk, m_qk, v_qk)   # [16,256], [16]
    wv_f, tv_f = fold_bn(w_v, g_v, b_v, m_v, v_v)          # [128,256], [128]
    wp_f, tp_f = fold_bn(w_p, g_p, b_p, m_p, v_p)          # [256,128], [256]

    # [128, 2, *]: partition dim first, C-half (or out-half) second.
    # wqkT replicated into 4 column groups of 32 (16 used + 16 zero) so the
    # S stage can row-pack 4 concurrent matmuls.
    wqkT_h = wqk_f.T.reshape(2, 128, KD).transpose(1, 0, 2)  # [128, 2, 16]
    wqkT = np.zeros((128, 2, 128), np.float32)
    for g in range(4):
        wqkT[:, :, 32 * g:32 * g + KD] = wqkT_h
    wqkT = np.ascontiguousarray(wqkT)
    wvT = np.ascontiguousarray(
        wv_f.T.reshape(2, 128, DH).transpose(1, 0, 2)).astype(ml_dtypes.bfloat16)
    wpT = np.ascontiguousarray(
        wp_f.T.reshape(128, 2, 128)).astype(np.float32)
    tqk = np.zeros((128, 1), np.float32)
    for g in range(4):
        tqk[32 * g:32 * g + KD, 0] = tqk_f
    tqk = np.ascontiguousarray(tqk)
    tv = tv_f.reshape(1, DH).astype(np.float32)
    tp = np.ascontiguousarray(tp_f.reshape(1, 2, 128)).astype(np.float32)

    xr = x.reshape(B, C, N).astype(np.float32)
    in_maps = []
    for c in range(N_CORES):
        b_, h_ = c // 2, c % 2
        # permute n so this core's half comes first
        if h_ == 0:
            xp = xr[b_]
        else:
            xp = np.concatenate([xr[b_][:, NSH:], xr[b_][:, :NSH]], axis=1)
        xp = np.ascontiguousarray(xp.reshape(2, 128, N).transpose(1, 0, 2))
        in_maps.append({
            "xf": xp.astype(np.float32),
            "xb": xp.astype(ml_dtypes.bfloat16),
            "wqkT": wqkT, "wvT": wvT, "wpT": wpT,
            "tqk": tqk, "tv": tv, "tp": tp,
            "on1": np.ones((1, 128), np.float32),
        })
    return in_maps


def assemble(results):
    """Per-core 'out' [2, 128, NSH] -> full [B, C, H, W]."""
    out = np.empty((B, C, N), np.float32)
    for c in range(N_CORES):
        b_, h_ = c // 2, c % 2
        o = results[c]["out"].reshape(C, NSH)
        out[b_][:, h_ * NSH:(h_ + 1) * NSH] = o
    return out.reshape(B, C, H, W)


def kernel(**inputs):
    from concourse.bass_utils import run_bass_kernel_spmd
    from concourse.bass_interp import get_hw_module

    inputs = {k: np.asarray(v) for k, v in inputs.items()}
    inputs.pop("key_v_input_reduction", None)  # unused by the reference
    nc = build_nc()
    in_maps = make_in_maps(**inputs)
    old_m = nc.m
    nc.m = get_hw_module(nc.m)
    try:
        res = run_bass_kernel_spmd(nc, in_maps, core_ids=list(range(N_CORES)))
    finally:
        nc.m = old_m
    return assemble(res.results)
